# revision 7
# baseline (speedup 1.0000x reference)
"""MetScore kernel for Trainium2 (8 NeuronCores, data-parallel over pixels).

Strategy
--------
All outputs derive from 26 per-timestep statistics, each a linear reduction
over the 460800 pixels of that timestep:
  corr moments: cnt, S1p, S1t, S2p, S2t, Spt         (over mc-masked pixels)
  per level l:  hits_l, p_tot_l, t_tot_l, mae_num_l  (threshold bins)

Device computes per-(core, partition) partial sums; host gathers, reduces and
applies the tiny final formulas.

Tricks used:
 * q = (p+1)*m = exp(LF*pn)*(mask>0.5): avoids expm1 and folds the mask so
   every threshold compare on q with thresholds shifted by +1 is already
   masked (masked pixels have q=0 < all thresholds).  The +1 offset in the
   moments is corrected on the host (S1p = Sq - cnt, etc).
 * cumulative indicators qge_l = (q >= thr_l+1): bins are differences of
   cumulative sums, applied host-side on the reduced scalars, so hits only
   needs the 13 tri-diagonal entries of the cumulative joint matrix
   C[i][j] = sum(qge_i * rge_j).
 * fused DVE ops (tensor_scalar / scalar_tensor_tensor / tensor_tensor_reduce)
   carry a free-dim accumulator, so each statistic costs one streaming pass.
 * exp / abs run on the scalar (ACT) engine, overlapped with DVE.
"""

import numpy as np

import concourse.bacc as bacc
import concourse.bass as bass
import concourse.mybir as mybir
import concourse.tile as tile
from concourse.bass_utils import run_bass_kernel_spmd

N_CORES = 8
T = 20
B = 2
HW_ = 480 * 480
NPX = B * HW_            # 460800 pixels per timestep
PARTS = 128
FREE_TOTAL = NPX // PARTS   # 3600
FREE = FREE_TOTAL // N_CORES  # 450 per core

LOG_FACTOR = float(np.log(30.0 + 1.0))
THR_S = [1.1, 2.0, 3.0, 6.0, 9.0]   # thresholds + 1 (compare against q=(p+1)m)
LEVEL_WEIGHTS = np.array([0.1, 0.1, 0.2, 0.25, 0.35], dtype=np.float64)
TIME_WEIGHTS = np.array([0.0075, 0.02, 0.03, 0.04, 0.05, 0.06, 0.07, 0.08,
                         0.09, 0.1, 0.09, 0.08, 0.07, 0.06, 0.05, 0.04, 0.03,
                         0.02, 0.0075, 0.005], dtype=np.float64)

# C pairs: tri-diagonal of the cumulative joint matrix (p-index i, t-index j)
C_PAIRS = [(0, 0), (1, 1), (2, 2), (3, 3), (4, 4),
           (0, 1), (1, 2), (2, 3), (3, 4),
           (1, 0), (2, 1), (3, 2), (4, 3)]

# accumulator column layout (per timestep)
I_CNT = 0
I_SQ, I_SR, I_SQQ, I_SRR, I_SQR = 1, 2, 3, 4, 5
I_PGE = 6     # 6..10
I_TGE = 11    # 11..15
I_D = 16      # 16..20
I_C = 21      # 21..33
NSTAT = 34


def _build_nc():
    f32 = mybir.dt.float32
    bf16 = mybir.dt.bfloat16
    AL = mybir.AluOpType
    AF = mybir.ActivationFunctionType

    nc = bacc.Bacc("TRN2", target_bir_lowering=False, debug=False)
    pn = nc.dram_tensor("pn", [PARTS, T, FREE], f32, kind="ExternalInput").ap()
    tn = nc.dram_tensor("tn", [PARTS, T, FREE], f32, kind="ExternalInput").ap()
    mk = nc.dram_tensor("mk", [PARTS, T, FREE], f32, kind="ExternalInput").ap()
    acc_out = nc.dram_tensor("acc", [PARTS, T * NSTAT], f32,
                             kind="ExternalOutput").ap()

    with tile.TileContext(nc) as tc:
        with tc.tile_pool(name="io", bufs=3) as iop, \
             tc.tile_pool(name="p32", bufs=2) as p32, \
             tc.tile_pool(name="pbf", bufs=2) as pbf, \
             tc.tile_pool(name="accp", bufs=1) as accp:
            acc = accp.tile([PARTS, T * NSTAT], f32, name="acc_sb")

            for t in range(T):
                def col(k, t=t):
                    return acc[:, t * NSTAT + k: t * NSTAT + k + 1]

                pnt = iop.tile([PARTS, FREE], f32, tag="pnt", name=f"pnt{t}")
                tnt = iop.tile([PARTS, FREE], f32, tag="tnt", name=f"tnt{t}")
                mkt = iop.tile([PARTS, FREE], f32, tag="mkt", name=f"mkt{t}")
                nc.sync.dma_start(pnt[:], pn[:, t, :])
                nc.sync.dma_start(tnt[:], tn[:, t, :])
                nc.sync.dma_start(mkt[:], mk[:, t, :])

                # ACT: pe = exp(LF*pn), te = exp(LF*tn)
                pe = p32.tile([PARTS, FREE], f32, tag="pe", name=f"pe{t}")
                te = p32.tile([PARTS, FREE], f32, tag="te", name=f"te{t}")
                nc.scalar.activation(pe[:], pnt[:], AF.Exp, scale=LOG_FACTOR)
                nc.scalar.activation(te[:], tnt[:], AF.Exp, scale=LOG_FACTOR)

                # m = (mask > 0.5); q = m * pe ; r = m * te
                m = p32.tile([PARTS, FREE], f32, tag="m", name=f"m{t}")
                nc.vector.tensor_scalar(m[:], mkt[:], 0.5, None, AL.is_gt)
                q = p32.tile([PARTS, FREE], f32, tag="q", name=f"q{t}")
                r = p32.tile([PARTS, FREE], f32, tag="r", name=f"r{t}")
                nc.vector.tensor_tensor(q[:], m[:], pe[:], AL.mult)
                nc.vector.tensor_tensor(r[:], m[:], te[:], AL.mult)

                # cumulative indicators (bf16) + their sums (Pge/Tge).
                # rge planes are reused by the C block; qge only l=0 (for mc).
                rge = []
                qge0 = None
                for l in range(5):
                    qg = pbf.tile([PARTS, FREE], bf16, tag=f"qge{l}",
                                  name=f"qge{l}_{t}")
                    rg = pbf.tile([PARTS, FREE], bf16, tag=f"rge{l}",
                                  name=f"rge{l}_{t}")
                    nc.vector.tensor_scalar(qg[:], q[:], THR_S[l], None,
                                            AL.is_ge, AL.add,
                                            accum_out=col(I_PGE + l))
                    nc.vector.tensor_scalar(rg[:], r[:], THR_S[l], None,
                                            AL.is_ge, AL.add,
                                            accum_out=col(I_TGE + l))
                    rge.append(rg)
                    if l == 0:
                        qge0 = qg

                # mc = qge0 OR rge0 (max), cnt = sum(mc)
                mc = p32.tile([PARTS, FREE], f32, tag="mc", name=f"mc{t}")
                nc.vector.scalar_tensor_tensor(
                    mc[:], qge0[:], 1.0, rge[0][:], AL.mult, AL.max,
                    accum_out=col(I_CNT))

                # moments: u = q*mc, v = r*mc, then u*q, v*r, u*r
                u = p32.tile([PARTS, FREE], f32, tag="u", name=f"u{t}")
                v = p32.tile([PARTS, FREE], f32, tag="v", name=f"v{t}")
                nc.vector.scalar_tensor_tensor(
                    u[:], q[:], 1.0, mc[:], AL.mult, AL.mult,
                    accum_out=col(I_SQ))
                nc.vector.scalar_tensor_tensor(
                    v[:], r[:], 1.0, mc[:], AL.mult, AL.mult,
                    accum_out=col(I_SR))
                scr = p32.tile([PARTS, FREE], f32, tag="scr", name=f"scr{t}")
                nc.vector.scalar_tensor_tensor(
                    scr[:], u[:], 1.0, q[:], AL.mult, AL.mult,
                    accum_out=col(I_SQQ))
                scr2 = p32.tile([PARTS, FREE], f32, tag="scr2", name=f"sc2{t}")
                nc.vector.scalar_tensor_tensor(
                    scr2[:], v[:], 1.0, r[:], AL.mult, AL.mult,
                    accum_out=col(I_SRR))
                scr3 = p32.tile([PARTS, FREE], f32, tag="scr3", name=f"sc3{t}")
                nc.vector.scalar_tensor_tensor(
                    scr3[:], u[:], 1.0, r[:], AL.mult, AL.mult,
                    accum_out=col(I_SQR))

                # d = |q - r| (sub on DVE, abs on ACT)
                dd = p32.tile([PARTS, FREE], f32, tag="dd", name=f"dd{t}")
                nc.vector.tensor_tensor(dd[:], q[:], r[:], AL.subtract)
                d = p32.tile([PARTS, FREE], f32, tag="d", name=f"d{t}")
                nc.scalar.activation(d[:], dd[:], AF.Abs)

                # D_l = sum((r >= thr_l) * d)
                for l in range(5):
                    sd = p32.tile([PARTS, FREE], f32, tag="sd", name=f"sd{l}_{t}")
                    nc.vector.scalar_tensor_tensor(
                        sd[:], r[:], THR_S[l], d[:], AL.is_ge, AL.mult,
                        accum_out=col(I_D + l))

                # C[i][j] = sum((q >= thr_i) * rge_j), tri-diagonal entries
                for k, (i, j) in enumerate(C_PAIRS):
                    sb = pbf.tile([PARTS, FREE], bf16, tag="sb", name=f"sb{k}_{t}")
                    nc.vector.scalar_tensor_tensor(
                        sb[:], q[:], THR_S[i], rge[j][:], AL.is_ge, AL.mult,
                        accum_out=col(I_C + k))

            nc.sync.dma_start(acc_out[:, :], acc[:])
    nc.compile()
    return nc


_NC_CACHE = {}


def _get_nc():
    if "nc" not in _NC_CACHE:
        _NC_CACHE["nc"] = _build_nc()
    return _NC_CACHE["nc"]


def shard_inputs(pred_norm, target_norm, mask):
    """Full [2,20,480,480] f32 inputs -> per-core dicts of [128, T, 450]."""
    in_maps = [dict() for _ in range(N_CORES)]
    for name, x in (("pn", pred_norm), ("tn", target_norm), ("mk", mask)):
        x = np.asarray(x, dtype=np.float32)
        # pixel index = b*HW + h*W + w ; [T, 128, 3600] pixel-major
        xt = np.transpose(x, (1, 0, 2, 3)).reshape(T, PARTS, FREE_TOTAL)
        for c in range(N_CORES):
            shard = np.ascontiguousarray(
                np.transpose(xt[:, :, c * FREE:(c + 1) * FREE], (1, 0, 2)))
            in_maps[c][name] = shard
    return in_maps


def stats_from_accs(accs):
    """accs: list of [128, T*NSTAT] f32 per core -> dict of per-ts stats."""
    a = np.stack([np.asarray(x, dtype=np.float64) for x in accs])  # [8,128,680]
    s = a.sum(axis=(0, 1)).reshape(T, NSTAT)                       # [T, 34]

    cnt = s[:, I_CNT]
    Sq, Sr = s[:, I_SQ], s[:, I_SR]
    Sqq, Srr, Sqr = s[:, I_SQQ], s[:, I_SRR], s[:, I_SQR]
    Pge = s[:, I_PGE:I_PGE + 5]
    Tge = s[:, I_TGE:I_TGE + 5]
    D = s[:, I_D:I_D + 5]
    C = {pair: s[:, I_C + k] for k, pair in enumerate(C_PAIRS)}

    # undo the +1 shift of q = (p+1)*m relative to p
    S1p = Sq - cnt
    S1t = Sr - cnt
    S2p = Sqq - 2.0 * Sq + cnt
    S2t = Srr - 2.0 * Sr + cnt
    Spt = Sqr - Sq - Sr + cnt

    hits = np.zeros((T, 5))
    for l in range(4):
        hits[:, l] = (C[(l, l)] - C[(l, l + 1)] - C[(l + 1, l)]
                      + C[(l + 1, l + 1)])
    hits[:, 4] = C[(4, 4)]

    def cum_to_bin(X):
        out = np.zeros((T, 5))
        out[:, :4] = X[:, :4] - X[:, 1:]
        out[:, 4] = X[:, 4]
        return out

    p_tot = cum_to_bin(Pge)
    t_tot = cum_to_bin(Tge)
    mae_num = cum_to_bin(D)
    return dict(cnt=cnt, S1p=S1p, S1t=S1t, S2p=S2p, S2t=S2t, Spt=Spt,
                hits=hits, p_tot=p_tot, t_tot=t_tot, mae_num=mae_num)


def finalize(st):
    """Replicate the reference's final formulas from the reduced stats."""
    cnt = st["cnt"]
    safe_cnt = np.maximum(cnt, 1.0)
    num = st["Spt"] - st["S1p"] * st["S1t"] / safe_cnt
    varp = np.maximum(st["S2p"] - st["S1p"] ** 2 / safe_cnt, 0.0)
    vart = np.maximum(st["S2t"] - st["S1t"] ** 2 / safe_cnt, 0.0)
    den = np.sqrt(varp * vart)
    r_time = np.where(cnt > 0, np.clip(num / (den + 1e-6), -1.0, 1.0), 0.0)

    hits, p_tot, t_tot = st["hits"], st["p_tot"], st["t_tot"]
    ts_mat = hits / (p_tot + t_tot - hits + 1e-8)
    mae_mat = np.where(t_tot > 0, st["mae_num"] / np.maximum(t_tot, 1.0), 0.0)

    term_corr = np.sqrt(np.exp(r_time - 1.0))
    term_mae = np.sqrt(np.exp(-mae_mat / 100.0))
    score_time = term_corr * (LEVEL_WEIGHTS[None, :] * ts_mat * term_mae).sum(-1)
    total = (score_time * TIME_WEIGHTS).sum()

    f = np.float32
    return (np.asarray(total, dtype=f),
            score_time.astype(f), r_time.astype(f),
            ts_mat.astype(f), mae_mat.astype(f),
            ts_mat.mean(0).astype(f), mae_mat.mean(0).astype(f))


def kernel(pred_norm, target_norm, mask):
    nc = _get_nc()
    in_maps = shard_inputs(pred_norm, target_norm, mask)
    res = run_bass_kernel_spmd(nc, in_maps, core_ids=list(range(N_CORES)))
    accs = [res.results[c]["acc"] for c in range(N_CORES)]
    return finalize(stats_from_accs(accs))


# revision 11
# speedup vs baseline: 261.1992x; 261.1992x over previous
"""MetScore kernel for Trainium2 (8 NeuronCores, data-parallel over pixels).

All outputs derive from 34 per-timestep statistics, each a linear reduction
over the 460800 pixels of that timestep (corr moments + per-level bin stats).
The pixel axis of each timestep is split into 4 quarters -> 80 (ts, quarter)
units of [128 partitions x 900 pixels]; core c processes units 10c..10c+9.
Device computes per-partition partial sums; host gathers and finishes.

Tricks:
 * q = (p+1)*m = exp(LF*pn)*(mask>0.5): avoids expm1, folds the mask into
   every threshold compare (thresholds shifted by +1); the +1 offset in the
   moments is corrected on the host.
 * cumulative indicators qge_l = (q >= thr_l+1): per-bin stats are
   differences of cumulative sums (host-side); hits needs only the 13
   tri-diagonal entries of C[i][j] = sum(qge_i * rge_j).
 * fused accumulators: tensor_scalar / scalar_tensor_tensor carry a free
   per-partition sum; ACT (scalar engine) computes exp/abs/squares and
   carries accumulators for the C products, balancing the two engines.
"""

import numpy as np

import concourse.bacc as bacc
import concourse.bass as bass
import concourse.mybir as mybir
import concourse.tile as tile
from concourse.bass_utils import run_bass_kernel_spmd

N_CORES = 8
T = 20
B = 2
NPX = B * 480 * 480          # 460800 pixels per timestep
PARTS = 128
FREE_TOTAL = NPX // PARTS    # 3600
QUARTERS = 4
FREE = FREE_TOTAL // QUARTERS    # 900
UNITS = T * QUARTERS             # 80
UNITS_PER_CORE = UNITS // N_CORES  # 10

LOG_FACTOR = float(np.log(30.0 + 1.0))
THR_S = [1.1, 2.0, 3.0, 6.0, 9.0]   # thresholds + 1 (compare against q=(p+1)m)
LEVEL_WEIGHTS = np.array([0.1, 0.1, 0.2, 0.25, 0.35], dtype=np.float64)
TIME_WEIGHTS = np.array([0.0075, 0.02, 0.03, 0.04, 0.05, 0.06, 0.07, 0.08,
                         0.09, 0.1, 0.09, 0.08, 0.07, 0.06, 0.05, 0.04, 0.03,
                         0.02, 0.0075, 0.005], dtype=np.float64)

C_PAIRS = [(0, 0), (1, 1), (2, 2), (3, 3), (4, 4),
           (0, 1), (1, 2), (2, 3), (3, 4),
           (1, 0), (2, 1), (3, 2), (4, 3)]

# accumulator column layout (per unit)
I_CNT = 0
I_SQ, I_SR, I_SQQ, I_SRR, I_SQR = 1, 2, 3, 4, 5
I_PGE = 6     # 6..10
I_TGE = 11    # 11..15
I_D = 16      # 16..20
I_C = 21      # 21..33
NSTAT = 34


def _build_nc(reps=1):
    """Build the Bass module. reps>1 wraps the body in a device-side loop
    over identical work (timing instrument only)."""
    f32 = mybir.dt.float32
    bf16 = mybir.dt.bfloat16
    AL = mybir.AluOpType
    AF = mybir.ActivationFunctionType

    nc = bacc.Bacc("TRN2", target_bir_lowering=False, debug=False)
    U = UNITS_PER_CORE
    pn = nc.dram_tensor("pn", [PARTS, U, FREE], f32, kind="ExternalInput").ap()
    tn = nc.dram_tensor("tn", [PARTS, U, FREE], f32, kind="ExternalInput").ap()
    mk = nc.dram_tensor("mk", [PARTS, U, FREE], f32, kind="ExternalInput").ap()
    acc_out = nc.dram_tensor("acc", [PARTS, 2 * U * NSTAT], f32,
                             kind="ExternalOutput").ap()

    with tile.TileContext(nc) as tc:
        with tc.tile_pool(name="io", bufs=2) as iop, \
             tc.tile_pool(name="p32", bufs=2) as p32, \
             tc.tile_pool(name="pbf", bufs=3) as pbf, \
             tc.tile_pool(name="accp", bufs=1) as accp:
            acc = accp.tile([PARTS, U * NSTAT], f32, name="acc_sb")
            acc2 = accp.tile([PARTS, U * NSTAT], f32, name="acc2_sb")
            nc.vector.memset(acc[:], 0.0)
            nc.scalar.memzero(acc2[:])

            import contextlib
            loop_ctx = tc.For_i(0, reps, 1) if reps > 1 else \
                contextlib.nullcontext()
            with loop_ctx:
                for t in range(U):
                    def col(k, t=t):
                        return acc[:, t * NSTAT + k: t * NSTAT + k + 1]

                    def col2(k, t=t):
                        return acc2[:, t * NSTAT + k: t * NSTAT + k + 1]

                    pnt = iop.tile([PARTS, FREE], f32, tag="pnt", name=f"pnt{t}")
                    tnt = iop.tile([PARTS, FREE], f32, tag="tnt", name=f"tnt{t}")
                    mkt = iop.tile([PARTS, FREE], f32, tag="mkt", name=f"mkt{t}")
                    nc.sync.dma_start(pnt[:], pn[:, t, :])
                    nc.sync.dma_start(tnt[:], tn[:, t, :])
                    nc.sync.dma_start(mkt[:], mk[:, t, :])

                    # ACT: pe = exp(LF*pn), te = exp(LF*tn)
                    pe = p32.tile([PARTS, FREE], f32, tag="pe", name=f"pe{t}")
                    te = p32.tile([PARTS, FREE], f32, tag="te", name=f"te{t}")
                    nc.scalar.activation(pe[:], pnt[:], AF.Exp, scale=LOG_FACTOR)
                    nc.scalar.activation(te[:], tnt[:], AF.Exp, scale=LOG_FACTOR)

                    # m = (mask > 0.5); q = m * pe ; r = m * te
                    m = p32.tile([PARTS, FREE], f32, tag="m", name=f"m{t}")
                    nc.vector.tensor_scalar(m[:], mkt[:], 0.5, None, AL.is_gt)
                    q = p32.tile([PARTS, FREE], f32, tag="q", name=f"q{t}")
                    r = p32.tile([PARTS, FREE], f32, tag="r", name=f"r{t}")
                    nc.vector.tensor_tensor(q[:], m[:], pe[:], AL.mult)
                    nc.vector.tensor_tensor(r[:], m[:], te[:], AL.mult)

                    # cumulative indicators (bf16 planes) + Pge/Tge accums
                    qge = []
                    rge = []
                    for l in range(5):
                        qg = pbf.tile([PARTS, FREE], bf16, tag=f"qge{l}",
                                      name=f"qge{l}_{t}")
                        rg = pbf.tile([PARTS, FREE], bf16, tag=f"rge{l}",
                                      name=f"rge{l}_{t}")
                        nc.vector.tensor_scalar(qg[:], q[:], THR_S[l], None,
                                                AL.is_ge, AL.add,
                                                accum_out=col(I_PGE + l))
                        nc.vector.tensor_scalar(rg[:], r[:], THR_S[l], None,
                                                AL.is_ge, AL.add,
                                                accum_out=col(I_TGE + l))
                        qge.append(qg)
                        rge.append(rg)

                    # mc = qge0 OR rge0 (max), cnt = sum(mc)
                    mc = p32.tile([PARTS, FREE], f32, tag="mc", name=f"mc{t}")
                    nc.vector.scalar_tensor_tensor(
                        mc[:], qge[0][:], 1.0, rge[0][:], AL.mult, AL.max,
                        accum_out=col(I_CNT))

                    # moments: u = q*mc (Sq), v = r*mc (Sr) on DVE;
                    # Sqq = sum(u^2), Srr = sum(v^2) on ACT; Sqr = sum(u*r) DVE
                    u = p32.tile([PARTS, FREE], f32, tag="u", name=f"u{t}")
                    v = p32.tile([PARTS, FREE], f32, tag="v", name=f"v{t}")
                    nc.vector.scalar_tensor_tensor(
                        u[:], q[:], 1.0, mc[:], AL.mult, AL.mult,
                        accum_out=col(I_SQ))
                    nc.vector.scalar_tensor_tensor(
                        v[:], r[:], 1.0, mc[:], AL.mult, AL.mult,
                        accum_out=col(I_SR))
                    squ = p32.tile([PARTS, FREE], f32, tag="squ", name=f"squ{t}")
                    sqv = p32.tile([PARTS, FREE], f32, tag="sqv", name=f"sqv{t}")
                    nc.scalar.activation(squ[:], u[:], AF.Square,
                                         accum_out=col2(I_SQQ))
                    nc.scalar.activation(sqv[:], v[:], AF.Square,
                                         accum_out=col2(I_SRR))
                    scr3 = p32.tile([PARTS, FREE], f32, tag="scr3", name=f"sc3{t}")
                    nc.vector.scalar_tensor_tensor(
                        scr3[:], u[:], 1.0, r[:], AL.mult, AL.mult,
                        accum_out=col(I_SQR))

                    # d = |q - r| (sub on DVE, abs on ACT)
                    dd = p32.tile([PARTS, FREE], f32, tag="dd", name=f"dd{t}")
                    nc.vector.tensor_tensor(dd[:], q[:], r[:], AL.subtract)
                    d = p32.tile([PARTS, FREE], f32, tag="d", name=f"d{t}")
                    nc.scalar.activation(d[:], dd[:], AF.Abs)

                    # D_l = sum((r >= thr_l) * d)   (DVE fused)
                    for l in range(5):
                        sd = p32.tile([PARTS, FREE], f32, tag="sd",
                                      name=f"sd{l}_{t}")
                        nc.vector.scalar_tensor_tensor(
                            sd[:], r[:], THR_S[l], d[:], AL.is_ge, AL.mult,
                            accum_out=col(I_D + l))

                    # C[i][j] = sum(qge_i * rge_j): product on DVE (bf16 2x),
                    # accumulation on ACT (copy with accum)
                    for k, (i, j) in enumerate(C_PAIRS):
                        cp = pbf.tile([PARTS, FREE], bf16, tag="cp",
                                      name=f"cp{k}_{t}")
                        nc.vector.tensor_tensor(cp[:], qge[i][:], rge[j][:],
                                                AL.mult)
                        ca = pbf.tile([PARTS, FREE], bf16, tag="ca",
                                      name=f"ca{k}_{t}")
                        nc.scalar.activation(ca[:], cp[:], AF.Copy,
                                             accum_out=col2(I_C + k))

            nc.sync.dma_start(acc_out[:, 0:U * NSTAT], acc[:])
            nc.sync.dma_start(acc_out[:, U * NSTAT:], acc2[:])
    nc.compile()
    return nc


_NC_CACHE = {}


def _get_nc():
    if "nc" not in _NC_CACHE:
        _NC_CACHE["nc"] = _build_nc()
    return _NC_CACHE["nc"]


def shard_inputs(pred_norm, target_norm, mask):
    """Full [2,20,480,480] f32 inputs -> per-core dicts of [128, 10, 900].

    Global unit g = 10*c + k maps to (ts = g//4, quarter = g%4)."""
    in_maps = [dict() for _ in range(N_CORES)]
    for name, x in (("pn", pred_norm), ("tn", target_norm), ("mk", mask)):
        x = np.asarray(x, dtype=np.float32)
        xt = np.transpose(x, (1, 0, 2, 3)).reshape(T, PARTS, FREE_TOTAL)
        xu = xt.reshape(T, PARTS, QUARTERS, FREE)
        xu = np.transpose(xu, (0, 2, 1, 3)).reshape(UNITS, PARTS, FREE)
        for c in range(N_CORES):
            shard = np.ascontiguousarray(np.transpose(
                xu[c * UNITS_PER_CORE:(c + 1) * UNITS_PER_CORE], (1, 0, 2)))
            in_maps[c][name] = shard
    return in_maps


def stats_from_accs(accs):
    """accs: list of [128, U*NSTAT] f32 per core -> dict of per-ts stats."""
    a = np.stack([np.asarray(x, dtype=np.float64) for x in accs])
    half = UNITS_PER_CORE * NSTAT
    a = a[:, :, :half] + a[:, :, half:]
    # [8, 128, 10, 34] -> per-unit stats -> merge quarters back into ts
    s_units = a.sum(axis=1).reshape(N_CORES * UNITS_PER_CORE, NSTAT)
    s = s_units.reshape(T, QUARTERS, NSTAT).sum(axis=1)   # [T, 34]

    cnt = s[:, I_CNT]
    Sq, Sr = s[:, I_SQ], s[:, I_SR]
    Sqq, Srr, Sqr = s[:, I_SQQ], s[:, I_SRR], s[:, I_SQR]
    Pge = s[:, I_PGE:I_PGE + 5]
    Tge = s[:, I_TGE:I_TGE + 5]
    D = s[:, I_D:I_D + 5]
    C = {pair: s[:, I_C + k] for k, pair in enumerate(C_PAIRS)}

    # undo the +1 shift of q = (p+1)*m relative to p
    S1p = Sq - cnt
    S1t = Sr - cnt
    S2p = Sqq - 2.0 * Sq + cnt
    S2t = Srr - 2.0 * Sr + cnt
    Spt = Sqr - Sq - Sr + cnt

    hits = np.zeros((T, 5))
    for l in range(4):
        hits[:, l] = (C[(l, l)] - C[(l, l + 1)] - C[(l + 1, l)]
                      + C[(l + 1, l + 1)])
    hits[:, 4] = C[(4, 4)]

    def cum_to_bin(X):
        out = np.zeros((T, 5))
        out[:, :4] = X[:, :4] - X[:, 1:]
        out[:, 4] = X[:, 4]
        return out

    p_tot = cum_to_bin(Pge)
    t_tot = cum_to_bin(Tge)
    mae_num = cum_to_bin(D)
    return dict(cnt=cnt, S1p=S1p, S1t=S1t, S2p=S2p, S2t=S2t, Spt=Spt,
                hits=hits, p_tot=p_tot, t_tot=t_tot, mae_num=mae_num)


def finalize(st):
    """Replicate the reference's final formulas from the reduced stats."""
    cnt = st["cnt"]
    safe_cnt = np.maximum(cnt, 1.0)
    num = st["Spt"] - st["S1p"] * st["S1t"] / safe_cnt
    varp = np.maximum(st["S2p"] - st["S1p"] ** 2 / safe_cnt, 0.0)
    vart = np.maximum(st["S2t"] - st["S1t"] ** 2 / safe_cnt, 0.0)
    den = np.sqrt(varp * vart)
    r_time = np.where(cnt > 0, np.clip(num / (den + 1e-6), -1.0, 1.0), 0.0)

    hits, p_tot, t_tot = st["hits"], st["p_tot"], st["t_tot"]
    ts_mat = hits / (p_tot + t_tot - hits + 1e-8)
    mae_mat = np.where(t_tot > 0, st["mae_num"] / np.maximum(t_tot, 1.0), 0.0)

    term_corr = np.sqrt(np.exp(r_time - 1.0))
    term_mae = np.sqrt(np.exp(-mae_mat / 100.0))
    score_time = term_corr * (LEVEL_WEIGHTS[None, :] * ts_mat * term_mae).sum(-1)
    total = (score_time * TIME_WEIGHTS).sum()

    f = np.float32
    return (np.asarray(total, dtype=f),
            score_time.astype(f), r_time.astype(f),
            ts_mat.astype(f), mae_mat.astype(f),
            ts_mat.mean(0).astype(f), mae_mat.mean(0).astype(f))


def kernel(pred_norm, target_norm, mask):
    nc = _get_nc()
    in_maps = shard_inputs(pred_norm, target_norm, mask)
    res = run_bass_kernel_spmd(nc, in_maps, core_ids=list(range(N_CORES)))
    accs = [res.results[c]["acc"] for c in range(N_CORES)]
    return finalize(stats_from_accs(accs))


# revision 14
# speedup vs baseline: 272.6826x; 1.0440x over previous
"""MetScore kernel for Trainium2 (8 NeuronCores, data-parallel over pixels).

All outputs derive from 34 per-timestep statistics, each a linear reduction
over the 460800 pixels of that timestep (corr moments + per-level bin stats).
The pixel axis of each timestep is split into 4 quarters -> 80 (ts, quarter)
units of [128 partitions x 900 pixels]; core c processes units 10c..10c+9.
Device computes per-partition partial sums; host gathers and finishes.

Tricks:
 * q = (p+1)*m = exp(LF*pn)*(mask>0.5): avoids expm1, folds the mask into
   every threshold compare (thresholds shifted by +1); the +1 offset in the
   moments is corrected on the host.
 * cumulative indicators qge_l = (q >= thr_l+1): per-bin stats are
   differences of cumulative sums (host-side); hits needs only the 13
   tri-diagonal entries of C[i][j] = sum(qge_i * rge_j).
 * fused accumulators: tensor_scalar / scalar_tensor_tensor carry a free
   per-partition sum; ACT (scalar engine) computes exp/abs/squares and
   carries accumulators for the C products, balancing the two engines.
"""

import os
os.environ.setdefault("NEURON_RT_RESET_CORES", "1")

import numpy as np

import concourse.bacc as bacc
import concourse.bass as bass
import concourse.mybir as mybir
import concourse.tile as tile
from concourse.bass_utils import run_bass_kernel_spmd

N_CORES = 8
T = 20
B = 2
NPX = B * 480 * 480          # 460800 pixels per timestep
PARTS = 128
FREE_TOTAL = NPX // PARTS    # 3600
QUARTERS = 2
FREE = FREE_TOTAL // QUARTERS    # 900
UNITS = T * QUARTERS             # 80
UNITS_PER_CORE = UNITS // N_CORES  # 10

LOG_FACTOR = float(np.log(30.0 + 1.0))
THR_S = [1.1, 2.0, 3.0, 6.0, 9.0]   # thresholds + 1 (compare against q=(p+1)m)
LEVEL_WEIGHTS = np.array([0.1, 0.1, 0.2, 0.25, 0.35], dtype=np.float64)
TIME_WEIGHTS = np.array([0.0075, 0.02, 0.03, 0.04, 0.05, 0.06, 0.07, 0.08,
                         0.09, 0.1, 0.09, 0.08, 0.07, 0.06, 0.05, 0.04, 0.03,
                         0.02, 0.0075, 0.005], dtype=np.float64)

C_PAIRS = [(0, 0), (1, 1), (2, 2), (3, 3), (4, 4),
           (0, 1), (1, 2), (2, 3), (3, 4),
           (1, 0), (2, 1), (3, 2), (4, 3)]

# accumulator column layout (per unit)
I_CNT = 0
I_SQ, I_SR, I_SQQ, I_SRR, I_SQR = 1, 2, 3, 4, 5
I_PGE = 6     # 6..10
I_TGE = 11    # 11..15
I_D = 16      # 16..20
I_C = 21      # 21..33
NSTAT = 34


def _build_nc(reps=1):
    """Build the Bass module. reps>1 wraps the body in a device-side loop
    over identical work (timing instrument only)."""
    f32 = mybir.dt.float32
    bf16 = mybir.dt.bfloat16
    AL = mybir.AluOpType
    AF = mybir.ActivationFunctionType

    nc = bacc.Bacc("TRN2", target_bir_lowering=False, debug=False)
    U = UNITS_PER_CORE
    pn = nc.dram_tensor("pn", [PARTS, U, FREE], f32, kind="ExternalInput").ap()
    tn = nc.dram_tensor("tn", [PARTS, U, FREE], f32, kind="ExternalInput").ap()
    mk = nc.dram_tensor("mk", [PARTS, U, FREE], f32, kind="ExternalInput").ap()
    acc_out = nc.dram_tensor("acc", [PARTS, 2 * U * NSTAT], f32,
                             kind="ExternalOutput").ap()

    with tile.TileContext(nc) as tc:
        with tc.tile_pool(name="io", bufs=2) as iop, \
             tc.tile_pool(name="p32", bufs=1) as p32, \
             tc.tile_pool(name="pbf", bufs=1) as pbf, \
             tc.tile_pool(name="accp", bufs=1) as accp:
            acc = accp.tile([PARTS, U * NSTAT], f32, name="acc_sb")
            acc2 = accp.tile([PARTS, U * NSTAT], f32, name="acc2_sb")
            nc.vector.memset(acc[:], 0.0)
            nc.scalar.memzero(acc2[:])

            import contextlib
            loop_ctx = tc.For_i(0, reps, 1) if reps > 1 else \
                contextlib.nullcontext()
            with loop_ctx:
                for t in range(U):
                    def col(k, t=t):
                        return acc[:, t * NSTAT + k: t * NSTAT + k + 1]

                    def col2(k, t=t):
                        return acc2[:, t * NSTAT + k: t * NSTAT + k + 1]

                    pnt = iop.tile([PARTS, FREE], f32, tag="pnt", name=f"pnt{t}")
                    tnt = iop.tile([PARTS, FREE], f32, tag="tnt", name=f"tnt{t}")
                    mkt = iop.tile([PARTS, FREE], f32, tag="mkt", name=f"mkt{t}")
                    nc.sync.dma_start(pnt[:], pn[:, t, :])
                    nc.sync.dma_start(tnt[:], tn[:, t, :])
                    nc.sync.dma_start(mkt[:], mk[:, t, :])

                    # ACT: pe = exp(LF*pn), te = exp(LF*tn)
                    pe = p32.tile([PARTS, FREE], f32, tag="pe", bufs=2, name=f"pe{t}")
                    te = p32.tile([PARTS, FREE], f32, tag="te", bufs=2, name=f"te{t}")
                    nc.scalar.activation(pe[:], pnt[:], AF.Exp, scale=LOG_FACTOR)
                    nc.scalar.activation(te[:], tnt[:], AF.Exp, scale=LOG_FACTOR)

                    # m = (mask > 0.5); q = m * pe ; r = m * te
                    m = p32.tile([PARTS, FREE], f32, tag="m", name=f"m{t}")
                    nc.vector.tensor_scalar(m[:], mkt[:], 0.5, None, AL.is_gt)
                    q = p32.tile([PARTS, FREE], f32, tag="q", name=f"q{t}")
                    r = p32.tile([PARTS, FREE], f32, tag="r", name=f"r{t}")
                    nc.vector.tensor_tensor(q[:], m[:], pe[:], AL.mult)
                    nc.vector.tensor_tensor(r[:], m[:], te[:], AL.mult)

                    # cumulative indicators (bf16 planes) + Pge/Tge accums
                    qge = []
                    rge = []
                    for l in range(5):
                        qg = pbf.tile([PARTS, FREE], bf16, tag=f"qge{l}",
                                      name=f"qge{l}_{t}")
                        rg = pbf.tile([PARTS, FREE], bf16, tag=f"rge{l}",
                                      name=f"rge{l}_{t}")
                        nc.vector.tensor_scalar(qg[:], q[:], THR_S[l], None,
                                                AL.is_ge, AL.add,
                                                accum_out=col(I_PGE + l))
                        nc.vector.tensor_scalar(rg[:], r[:], THR_S[l], None,
                                                AL.is_ge, AL.add,
                                                accum_out=col(I_TGE + l))
                        qge.append(qg)
                        rge.append(rg)

                    # mc = qge0 OR rge0 (max), cnt = sum(mc)
                    mc = p32.tile([PARTS, FREE], f32, tag="mc", name=f"mc{t}")
                    nc.vector.scalar_tensor_tensor(
                        mc[:], qge[0][:], 1.0, rge[0][:], AL.mult, AL.max,
                        accum_out=col(I_CNT))

                    # moments: u = q*mc (Sq), v = r*mc (Sr) on DVE;
                    # Sqq = sum(u^2), Srr = sum(v^2) on ACT; Sqr = sum(u*r) DVE
                    u = p32.tile([PARTS, FREE], f32, tag="u", name=f"u{t}")
                    v = p32.tile([PARTS, FREE], f32, tag="v", name=f"v{t}")
                    nc.vector.scalar_tensor_tensor(
                        u[:], q[:], 1.0, mc[:], AL.mult, AL.mult,
                        accum_out=col(I_SQ))
                    nc.vector.scalar_tensor_tensor(
                        v[:], r[:], 1.0, mc[:], AL.mult, AL.mult,
                        accum_out=col(I_SR))
                    squ = p32.tile([PARTS, FREE], f32, tag="squ", name=f"squ{t}")
                    sqv = p32.tile([PARTS, FREE], f32, tag="squ", name=f"sqv{t}")
                    nc.scalar.activation(squ[:], u[:], AF.Square,
                                         accum_out=col2(I_SQQ))
                    nc.scalar.activation(sqv[:], v[:], AF.Square,
                                         accum_out=col2(I_SRR))
                    scr3 = p32.tile([PARTS, FREE], f32, tag="scrD", name=f"sc3{t}")
                    nc.vector.scalar_tensor_tensor(
                        scr3[:], u[:], 1.0, r[:], AL.mult, AL.mult,
                        accum_out=col(I_SQR))

                    # d = |q - r| (sub on DVE, abs on ACT)
                    dd = p32.tile([PARTS, FREE], f32, tag="dd", name=f"dd{t}")
                    nc.vector.tensor_tensor(dd[:], q[:], r[:], AL.subtract)
                    d = p32.tile([PARTS, FREE], f32, tag="d", name=f"d{t}")
                    nc.scalar.activation(d[:], dd[:], AF.Abs)

                    # D_l = sum((r >= thr_l) * d)   (DVE fused)
                    for l in range(5):
                        sd = p32.tile([PARTS, FREE], f32, tag="scrD",
                                      name=f"sd{l}_{t}")
                        nc.vector.scalar_tensor_tensor(
                            sd[:], r[:], THR_S[l], d[:], AL.is_ge, AL.mult,
                            accum_out=col(I_D + l))

                    # C[i][j] = sum(qge_i * rge_j): product on DVE (bf16 2x),
                    # accumulation on ACT (copy with accum)
                    for k, (i, j) in enumerate(C_PAIRS):
                        cp = pbf.tile([PARTS, FREE], bf16, tag="cp", bufs=3,
                                      name=f"cp{k}_{t}")
                        nc.vector.tensor_tensor(cp[:], qge[i][:], rge[j][:],
                                                AL.mult)
                        ca = pbf.tile([PARTS, FREE], bf16, tag="ca",
                                      name=f"ca{k}_{t}")
                        nc.scalar.activation(ca[:], cp[:], AF.Copy,
                                             accum_out=col2(I_C + k))

            nc.sync.dma_start(acc_out[:, 0:U * NSTAT], acc[:])
            nc.sync.dma_start(acc_out[:, U * NSTAT:], acc2[:])
    nc.compile()
    return nc


_NC_CACHE = {}


def _get_nc():
    if "nc" not in _NC_CACHE:
        _NC_CACHE["nc"] = _build_nc()
    return _NC_CACHE["nc"]


def shard_inputs(pred_norm, target_norm, mask):
    """Full [2,20,480,480] f32 inputs -> per-core dicts of [128, 10, 900].

    Global unit g = 10*c + k maps to (ts = g//4, quarter = g%4)."""
    in_maps = [dict() for _ in range(N_CORES)]
    for name, x in (("pn", pred_norm), ("tn", target_norm), ("mk", mask)):
        x = np.asarray(x, dtype=np.float32)
        xt = np.transpose(x, (1, 0, 2, 3)).reshape(T, PARTS, FREE_TOTAL)
        xu = xt.reshape(T, PARTS, QUARTERS, FREE)
        xu = np.transpose(xu, (0, 2, 1, 3)).reshape(UNITS, PARTS, FREE)
        for c in range(N_CORES):
            shard = np.ascontiguousarray(np.transpose(
                xu[c * UNITS_PER_CORE:(c + 1) * UNITS_PER_CORE], (1, 0, 2)))
            in_maps[c][name] = shard
    return in_maps


def stats_from_accs(accs):
    """accs: list of [128, U*NSTAT] f32 per core -> dict of per-ts stats."""
    a = np.stack([np.asarray(x, dtype=np.float64) for x in accs])
    half = UNITS_PER_CORE * NSTAT
    a = a[:, :, :half] + a[:, :, half:]
    # [8, 128, 10, 34] -> per-unit stats -> merge quarters back into ts
    s_units = a.sum(axis=1).reshape(N_CORES * UNITS_PER_CORE, NSTAT)
    s = s_units.reshape(T, QUARTERS, NSTAT).sum(axis=1)   # [T, 34]

    cnt = s[:, I_CNT]
    Sq, Sr = s[:, I_SQ], s[:, I_SR]
    Sqq, Srr, Sqr = s[:, I_SQQ], s[:, I_SRR], s[:, I_SQR]
    Pge = s[:, I_PGE:I_PGE + 5]
    Tge = s[:, I_TGE:I_TGE + 5]
    D = s[:, I_D:I_D + 5]
    C = {pair: s[:, I_C + k] for k, pair in enumerate(C_PAIRS)}

    # undo the +1 shift of q = (p+1)*m relative to p
    S1p = Sq - cnt
    S1t = Sr - cnt
    S2p = Sqq - 2.0 * Sq + cnt
    S2t = Srr - 2.0 * Sr + cnt
    Spt = Sqr - Sq - Sr + cnt

    hits = np.zeros((T, 5))
    for l in range(4):
        hits[:, l] = (C[(l, l)] - C[(l, l + 1)] - C[(l + 1, l)]
                      + C[(l + 1, l + 1)])
    hits[:, 4] = C[(4, 4)]

    def cum_to_bin(X):
        out = np.zeros((T, 5))
        out[:, :4] = X[:, :4] - X[:, 1:]
        out[:, 4] = X[:, 4]
        return out

    p_tot = cum_to_bin(Pge)
    t_tot = cum_to_bin(Tge)
    mae_num = cum_to_bin(D)
    return dict(cnt=cnt, S1p=S1p, S1t=S1t, S2p=S2p, S2t=S2t, Spt=Spt,
                hits=hits, p_tot=p_tot, t_tot=t_tot, mae_num=mae_num)


def finalize(st):
    """Replicate the reference's final formulas from the reduced stats."""
    cnt = st["cnt"]
    safe_cnt = np.maximum(cnt, 1.0)
    num = st["Spt"] - st["S1p"] * st["S1t"] / safe_cnt
    varp = np.maximum(st["S2p"] - st["S1p"] ** 2 / safe_cnt, 0.0)
    vart = np.maximum(st["S2t"] - st["S1t"] ** 2 / safe_cnt, 0.0)
    den = np.sqrt(varp * vart)
    r_time = np.where(cnt > 0, np.clip(num / (den + 1e-6), -1.0, 1.0), 0.0)

    hits, p_tot, t_tot = st["hits"], st["p_tot"], st["t_tot"]
    ts_mat = hits / (p_tot + t_tot - hits + 1e-8)
    mae_mat = np.where(t_tot > 0, st["mae_num"] / np.maximum(t_tot, 1.0), 0.0)

    term_corr = np.sqrt(np.exp(r_time - 1.0))
    term_mae = np.sqrt(np.exp(-mae_mat / 100.0))
    score_time = term_corr * (LEVEL_WEIGHTS[None, :] * ts_mat * term_mae).sum(-1)
    total = (score_time * TIME_WEIGHTS).sum()

    f = np.float32
    return (np.asarray(total, dtype=f),
            score_time.astype(f), r_time.astype(f),
            ts_mat.astype(f), mae_mat.astype(f),
            ts_mat.mean(0).astype(f), mae_mat.mean(0).astype(f))


def kernel(pred_norm, target_norm, mask):
    nc = _get_nc()
    in_maps = shard_inputs(pred_norm, target_norm, mask)
    res = run_bass_kernel_spmd(nc, in_maps, core_ids=list(range(N_CORES)))
    accs = [res.results[c]["acc"] for c in range(N_CORES)]
    return finalize(stats_from_accs(accs))
